# revision 3
# baseline (speedup 1.0000x reference)
"""ConvAttention Trainium2 kernel: 8-core SPMD (batch x seq-half sharding).

Per core (b = core//2, half = core%2): computes out[b, half*1024:(half+1)*1024, :]
and a1[b, :, half*1024:(half+1)*1024, :] (stored k-major as a1t[h, k, q]; host
transposes during unshard — pure layout glue, all math on device).

Device layout: channel-major activations. Scores computed transposed ([k, q])
so the softmax denominator falls out of the context matmul via an augmented
ones-column on V, and mask/scale/exp fuse into one ScalarE activation per
tile. Projections/dense in float32r (full PE rate), attention inner path bf16.
"""
import numpy as np

_CACHE = {}

B, L, D = 4, 2048, 768
H, S, KER = 6, 64, 9
DIM = H * S            # 384
HK = H * KER           # 54
LH = L // 2            # tokens per core
XQR = 1152             # xq rows (9 tiles of 128)
NCH = D // 128         # 6
C3 = DIM // 128        # 3
KC = L // 128          # 16
QJ = LH // 512         # 2


def _build():
    import concourse.bacc as bacc
    import concourse.tile as tile
    from concourse import mybir
    from contextlib import ExitStack

    f32 = mybir.dt.float32
    f32r = mybir.dt.float32r
    bf16 = mybir.dt.bfloat16
    AF = mybir.ActivationFunctionType
    MUL, ADD = mybir.AluOpType.mult, mybir.AluOpType.add

    nc = bacc.Bacc("TRN2", target_bir_lowering=False, debug=False, num_devices=8)

    xf_d = nc.dram_tensor("xf", [L, D], f32, kind="ExternalInput")
    xq_d = nc.dram_tensor("xq", [XQR, D], f32, kind="ExternalInput")
    mask_d = nc.dram_tensor("mask", [L], mybir.dt.int32, kind="ExternalInput")
    w_d = {}
    for nm in ["wq_w", "wk_w", "wv_w", "cc_pw", "co_w"]:
        w_d[nm] = nc.dram_tensor(nm, [D, DIM], f32r, kind="ExternalInput")
    dense_d = nc.dram_tensor("dense_w", [2 * DIM, D], f32r, kind="ExternalInput")
    ckw_d = nc.dram_tensor("ck_w", [DIM, HK], bf16, kind="ExternalInput")
    cc_dw_d = nc.dram_tensor("cc_dw", [KER, D], f32, kind="ExternalInput")
    b_d = {}
    for nm, n in [("wq_b", DIM), ("wk_b", DIM), ("wv_b", DIM), ("cc_b", DIM),
                  ("co_b", DIM), ("ck_b", HK), ("dense_b", D), ("gamma", D), ("beta", D)]:
        b_d[nm] = nc.dram_tensor(nm, [n], f32, kind="ExternalInput")
    ek_d = nc.dram_tensor("ek", [KER, C3, HK, 128], bf16, kind="ExternalInput")
    ssum_d = nc.dram_tensor("ssum", [HK, H], bf16, kind="ExternalInput")
    sbc_d = nc.dram_tensor("sbc", [H, HK], bf16, kind="ExternalInput")
    ident_d = nc.dram_tensor("ident", [128, 128], f32, kind="ExternalInput")
    identr_d = nc.dram_tensor("identr", [128, 128], f32r, kind="ExternalInput")

    a1t_d = nc.dram_tensor("a1t", [H, L, LH], f32, kind="ExternalOutput")
    out_d = nc.dram_tensor("outt", [D, LH], f32, kind="ExternalOutput")
    a1t_v = a1t_d.rearrange("h (kc p) q -> h p kc q", p=128)
    out_v = out_d.rearrange("(c p) t -> p c t", p=128)

    with tile.TileContext(nc) as tc, ExitStack() as top:
        pp = top.enter_context(tc.tile_pool(name="persist", bufs=1))

        kT = pp.tile([128, C3, L], bf16)
        qT = pp.tile([128, C3, LH], bf16)
        v_sb = pp.tile([128, KC, H, S + 1], bf16)
        localT = pp.tile([128, C3, LH], f32r)
        ctxT = pp.tile([128, C3, LH], f32r)
        xqT = pp.tile([128, NCH, 1040], f32r)
        ident = pp.tile([128, 128], f32)
        ident_r = pp.tile([128, 128], f32r)
        maskb = pp.tile([128, KC], f32)
        ones_row = pp.tile([1, 512], f32r)
        ones_col = pp.tile([128, 1], f32r)
        biases = pp.tile([128, 15], f32)
        ckb = pp.tile([HK, 1], f32)
        grow = pp.tile([1, D], f32r)
        brow = pp.tile([1, D], f32r)
        dbias = pp.tile([128, NCH], f32)
        eps_t = pp.tile([1, 1], f32)

        nc.sync.dma_start(ident[:], ident_d[:])
        nc.sync.dma_start(ident_r[:], identr_d[:])
        mi = pp.tile([128, KC], mybir.dt.int32)
        nc.sync.dma_start(mi[:], mask_d.rearrange("(kc p) -> p kc", p=128))
        mf = pp.tile([128, KC], f32)
        nc.vector.tensor_copy(mf[:], mi[:])
        nc.vector.tensor_scalar(maskb[:], mf[:], -10000.0, None, op0=MUL)
        of = pp.tile([1, 512], f32)
        nc.vector.memset(of[:], 1.0)
        nc.vector.tensor_copy(ones_row[:], of[:])
        oc_ = pp.tile([128, 1], f32)
        nc.vector.memset(oc_[:], 1.0)
        nc.vector.tensor_copy(ones_col[:], oc_[:])
        nc.vector.memset(v_sb[:, :, :, S], 1.0)
        for i, nm in enumerate(["wq_b", "wk_b", "wv_b", "cc_b", "co_b"]):
            nc.sync.dma_start(biases[:, 3 * i:3 * i + 3],
                              b_d[nm].rearrange("(c p) -> p c", p=128))
        nc.sync.dma_start(ckb[:], b_d["ck_b"][:, None])
        gf = pp.tile([1, 2 * D], f32)
        nc.sync.dma_start(gf[:, :D], b_d["gamma"][None, :])
        nc.sync.dma_start(gf[:, D:], b_d["beta"][None, :])
        nc.vector.tensor_copy(grow[:], gf[:, :D])
        nc.vector.tensor_copy(brow[:], gf[:, D:])
        nc.sync.dma_start(dbias[:], b_d["dense_b"].rearrange("(c p) -> p c", p=128))
        nc.vector.memset(eps_t[:], 1e-6)

        # ======== PHASE A1: xq transpose; xf transpose streamed -> kT, v ========
        with ExitStack() as ph:
            pa = ph.enter_context(tc.tile_pool(name="pa1", bufs=2))
            pw1 = ph.enter_context(tc.tile_pool(name="pw1", bufs=1))
            psA = ph.enter_context(tc.tile_pool(name="psA1", bufs=2, space="PSUM"))

            wk_r = pw1.tile([128, NCH, DIM], f32r, tag="wk")
            nc.sync.dma_start(wk_r[:], w_d["wk_w"].rearrange("(c p) d -> p c d", p=128))
            wv_r = pw1.tile([128, NCH, DIM], f32r, tag="wv")
            nc.sync.dma_start(wv_r[:], w_d["wv_w"].rearrange("(c p) d -> p c d", p=128))

            for tt in range(XQR // 128):
                xrow = pa.tile([128, D], f32, tag="xrow")
                nc.sync.dma_start(xrow[:], xq_d[tt * 128:(tt + 1) * 128, :])
                w = 128 if tt < 8 else 16
                for c in range(NCH):
                    pt = psA.tile([128, 128], f32, tag="tp")
                    nc.tensor.transpose(pt[:], xrow[:, c * 128:(c + 1) * 128], ident[:])
                    if c % 2:
                        nc.scalar.activation(xqT[:, c, tt * 128:tt * 128 + w],
                                             pt[:, :w], AF.Identity)
                    else:
                        nc.vector.tensor_copy(xqT[:, c, tt * 128:tt * 128 + w], pt[:, :w])

            for t4 in range(L // 512):
                xspan = pa.tile([128, NCH, 512], f32r, tag="xspan")
                for st in range(4):
                    tt = t4 * 4 + st
                    xrow = pa.tile([128, D], f32, tag="xrow")
                    nc.sync.dma_start(xrow[:], xf_d[tt * 128:(tt + 1) * 128, :])
                    for c in range(NCH):
                        pt = psA.tile([128, 128], f32, tag="tp")
                        nc.tensor.transpose(pt[:], xrow[:, c * 128:(c + 1) * 128], ident[:])
                        if c % 2:
                            nc.scalar.activation(xspan[:, c, st * 128:(st + 1) * 128],
                                                 pt[:], AF.Identity)
                        else:
                            nc.vector.tensor_copy(xspan[:, c, st * 128:(st + 1) * 128], pt[:])
                for co in range(C3):
                    ps = psA.tile([128, 512], f32, tag="ps1")
                    for ci in range(NCH):
                        nc.tensor.matmul(ps[:], wk_r[:, ci, co * 128:(co + 1) * 128],
                                         xspan[:, ci, :], start=(ci == 0), stop=(ci == NCH - 1))
                    nc.scalar.activation(kT[:, co, t4 * 512:(t4 + 1) * 512], ps[:],
                                         AF.Identity, bias=biases[:, 3 + co:4 + co])
                for st in range(4):
                    tt = t4 * 4 + st
                    ps = psA.tile([128, DIM], f32, tag="ps1")
                    for ci in range(NCH):
                        nc.tensor.matmul(ps[:, :DIM], xspan[:, ci, st * 128:(st + 1) * 128],
                                         wv_r[:, ci, :], start=(ci == 0), stop=(ci == NCH - 1))
                    nc.vector.tensor_copy(v_sb[:, tt, :, 0:S],
                                          ps[:, :DIM].rearrange("p (h s) -> p h s", h=H))

        # ======== PHASE A2: q, cox, depthwise, ccx, ck, lw, local ========
        with ExitStack() as ph:
            pa = ph.enter_context(tc.tile_pool(name="pa2", bufs=2))
            pw2 = ph.enter_context(tc.tile_pool(name="pw2", bufs=1))
            psA = ph.enter_context(tc.tile_pool(name="psA2", bufs=2, space="PSUM"))
            psY = ph.enter_context(tc.tile_pool(name="psY", bufs=2, space="PSUM"))

            wq_r = pw2.tile([128, NCH, DIM], f32r, tag="wq")
            nc.sync.dma_start(wq_r[:], w_d["wq_w"].rearrange("(c p) d -> p c d", p=128))
            co_r = pw2.tile([128, NCH, DIM], f32r, tag="co")
            nc.sync.dma_start(co_r[:], w_d["co_w"].rearrange("(c p) d -> p c d", p=128))
            pw_r = pw2.tile([128, NCH, DIM], f32r, tag="pw")
            nc.sync.dma_start(pw_r[:], w_d["cc_pw"].rearrange("(c p) d -> p c d", p=128))
            ck_r = pw2.tile([128, C3, HK], bf16, tag="ckw")
            nc.sync.dma_start(ck_r[:], ckw_d.rearrange("(c p) d -> p c d", p=128))
            ek_b = pw2.tile([HK, KER, C3, 128], bf16, tag="ek")
            nc.sync.dma_start(ek_b[:], ek_d.rearrange("k c p m -> p k c m"))
            ss_b = pw2.tile([HK, H], bf16, tag="ss")
            nc.sync.dma_start(ss_b[:], ssum_d[:])
            sb_b = pw2.tile([H, HK], bf16, tag="sb")
            nc.sync.dma_start(sb_b[:], sbc_d[:])
            dwf = pw2.tile([128, KER, NCH], f32, tag="dwf")
            nc.sync.dma_start(dwf[:], cc_dw_d.rearrange("j (c p) -> p j c", p=128))

            coxT = pw2.tile([128, C3, LH + 16], bf16, tag="coxT")
            yT = pw2.tile([128, NCH, LH], f32r, tag="yT")
            prod = pw2.tile([128, C3, LH], bf16, tag="prod")
            lwn = pw2.tile([HK, QJ, 512], bf16, tag="lwn")

            for co in range(C3):
                for j in range(QJ):
                    ps = psA.tile([128, 512], f32, tag="ps2")
                    for ci in range(NCH):
                        nc.tensor.matmul(ps[:], wq_r[:, ci, co * 128:(co + 1) * 128],
                                         xqT[:, ci, 4 + j * 512: 4 + j * 512 + 512],
                                         start=(ci == 0), stop=(ci == NCH - 1))
                    nc.scalar.activation(qT[:, co, j * 512:(j + 1) * 512], ps[:],
                                         AF.Identity, bias=biases[:, co:co + 1])
                for (c0, n) in ((0, 512), (512, 512), (1024, 16)):
                    ps = psA.tile([128, 512], f32, tag="ps2")
                    for ci in range(NCH):
                        nc.tensor.matmul(ps[:, :n], co_r[:, ci, co * 128:(co + 1) * 128],
                                         xqT[:, ci, c0:c0 + n],
                                         start=(ci == 0), stop=(ci == NCH - 1))
                    nc.scalar.activation(coxT[:, co, c0:c0 + n], ps[:, :n],
                                         AF.Identity, bias=biases[:, 12 + co:13 + co])
            for ci in range(NCH):
                dg = []
                for jj in range(KER):
                    d = pa.tile([128, 128], f32r, tag=f"diag{jj}")
                    nc.vector.tensor_scalar(d[:], ident[:], dwf[:, jj, ci:ci + 1], None,
                                            op0=MUL)
                    dg.append(d)
                for j in range(QJ):
                    ps = psY.tile([128, 512], f32, tag="ypsum")
                    for jj in range(KER):
                        nc.tensor.matmul(ps[:], dg[jj][:],
                                         xqT[:, ci, j * 512 + jj: j * 512 + jj + 512],
                                         start=(jj == 0), stop=(jj == KER - 1))
                    nc.scalar.activation(yT[:, ci, j * 512:(j + 1) * 512], ps[:], AF.Identity)
            for co in range(C3):
                for j in range(QJ):
                    ps = psA.tile([128, 512], f32, tag="ps2")
                    for ci in range(NCH):
                        nc.tensor.matmul(ps[:], pw_r[:, ci, co * 128:(co + 1) * 128],
                                         yT[:, ci, j * 512:(j + 1) * 512],
                                         start=(ci == 0), stop=(ci == NCH - 1))
                    cx = pa.tile([128, 512], bf16, tag="ccx")
                    nc.scalar.activation(cx[:], ps[:], AF.Identity,
                                         bias=biases[:, 9 + co:10 + co])
                    nc.vector.tensor_mul(prod[:, co, j * 512:(j + 1) * 512],
                                         qT[:, co, j * 512:(j + 1) * 512], cx[:])
            for j in range(QJ):
                ps = psA.tile([128, 512], f32, tag="ps2")
                for ci in range(C3):
                    nc.tensor.matmul(ps[:HK, :], ck_r[:, ci, :],
                                     prod[:, ci, j * 512:(j + 1) * 512],
                                     start=(ci == 0), stop=(ci == C3 - 1))
                elw = pa.tile([HK, 512], bf16, tag="elw")
                nc.scalar.activation(elw[:], ps[:HK, :], AF.Exp, bias=ckb[:])
                ps2 = psA.tile([H, 512], f32, tag="ps2b")
                nc.tensor.matmul(ps2[:], ss_b[:], elw[:], start=True, stop=True)
                rc = pa.tile([H, 512], f32, tag="lwr")
                nc.vector.reciprocal(rc[:], ps2[:])
                rcr = pa.tile([H, 512], bf16, tag="lwrr")
                nc.vector.tensor_copy(rcr[:], rc[:])
                ps3 = psA.tile([HK, 512], f32, tag="ps2b")
                nc.tensor.matmul(ps3[:], sb_b[:], rcr[:], start=True, stop=True)
                nc.vector.tensor_mul(lwn[:, j, :], elw[:], ps3[:])
            for ci in range(C3):
                for j in range(QJ):
                    acc = pa.tile([128, 512], bf16, tag="lacc")
                    tmp = pa.tile([128, 512], bf16, tag="ltmp")
                    for k in range(KER):
                        ps = psY.tile([128, 512], f32, tag="lwbp")
                        nc.tensor.matmul(ps[:], ek_b[:, k, ci, :], lwn[:, j, :],
                                         start=True, stop=True)
                        dst = acc if k == 0 else tmp
                        nc.vector.tensor_mul(dst[:],
                                             coxT[:, ci, j * 512 + k: j * 512 + k + 512],
                                             ps[:])
                        if k > 0:
                            nc.vector.tensor_add(acc[:], acc[:], tmp[:])
                    nc.vector.tensor_copy(localT[:, ci, j * 512:(j + 1) * 512], acc[:])

        # ======== PHASE B: attention ========
        with ExitStack() as ph:
            pb = ph.enter_context(tc.tile_pool(name="pb", bufs=2))
            pbs = ph.enter_context(tc.tile_pool(name="pbs", bufs=2))
            psS = ph.enter_context(tc.tile_pool(name="psS", bufs=2, space="PSUM"))
            psC = ph.enter_context(tc.tile_pool(name="psC", bufs=2, space="PSUM"))
            psB = ph.enter_context(tc.tile_pool(name="psB", bufs=2, space="PSUM"))
            for h in range(H):
                hp, hb = h // 2, (h % 2) * 64
                for j in range(QJ):
                    expT = pb.tile([128, KC, 512], bf16, tag="expT")
                    a1st = pb.tile([128, KC, 512], bf16, tag="a1st")
                    ctxp = psC.tile([S + 1, 512], f32, tag="ctx")
                    for kc in range(KC):
                        sc = psS.tile([128, 512], f32, tag="sc")
                        nc.tensor.matmul(sc[:], kT[hb:hb + 64, hp, kc * 128:(kc + 1) * 128],
                                         qT[hb:hb + 64, hp, j * 512:(j + 1) * 512],
                                         start=True, stop=True)
                        nc.scalar.activation(expT[:, kc, :], sc[:], AF.Exp,
                                             bias=maskb[:, kc:kc + 1], scale=0.125)
                        nc.tensor.matmul(ctxp[:], v_sb[:, kc, h, :], expT[:, kc, :],
                                         start=(kc == 0), stop=(kc == KC - 1))
                    zr = pbs.tile([1, 512], f32, tag="zr")
                    nc.vector.reciprocal(zr[:], ctxp[S:S + 1, :])
                    zrr = pbs.tile([1, 512], f32r, tag="zrr")
                    nc.vector.tensor_copy(zrr[:], zr[:])
                    bcp = psB.tile([128, 512], f32, tag="bc")
                    nc.tensor.matmul(bcp[:], ones_row[:, :128], zrr[:], start=True, stop=True)
                    bc = pbs.tile([128, 512], bf16, tag="bcs")
                    nc.scalar.activation(bc[:], bcp[:], AF.Identity)
                    ctmp = pbs.tile([S, 512], f32, tag="ctmp")
                    nc.vector.tensor_mul(ctmp[:], ctxp[0:S, :], bc[0:S, :])
                    nc.vector.tensor_scalar(
                        ctxT[hb:hb + 64, hp, j * 512:(j + 1) * 512], ctmp[:],
                        biases[hb:hb + 64, 6 + hp:7 + hp], None, op0=ADD)
                    nc.vector.tensor_mul(a1st[:], expT[:],
                                         bc[:, None, :].to_broadcast(expT.shape))
                    nc.gpsimd.dma_start(a1t_v[h][:, :, j * 512:(j + 1) * 512], a1st[:])

        # ======== PHASE C: dense + residual + layernorm ========
        with ExitStack() as ph:
            pcD = ph.enter_context(tc.tile_pool(name="pcD", bufs=1))
            pcw = ph.enter_context(tc.tile_pool(name="pcw", bufs=2))
            psD = ph.enter_context(tc.tile_pool(name="psD", bufs=2, space="PSUM"))
            psL = ph.enter_context(tc.tile_pool(name="psL", bufs=1, space="PSUM"))
            dn_r = pcD.tile([128, NCH, D], f32r)
            nc.sync.dma_start(dn_r[:], dense_d.rearrange("(c p) d -> p c d", p=128))
            hT = pcD.tile([128, NCH, LH], f32r)
            outT = pcD.tile([128, NCH, LH], f32)
            for j in range(QJ):
                for oc in range(NCH):
                    ps = psD.tile([128, 512], f32, tag="dns")
                    for ic in range(C3):
                        nc.tensor.matmul(ps[:], dn_r[:, ic, oc * 128:(oc + 1) * 128],
                                         ctxT[:, ic, j * 512:(j + 1) * 512],
                                         start=(ic == 0), stop=False)
                    for ic in range(C3):
                        nc.tensor.matmul(ps[:], dn_r[:, C3 + ic, oc * 128:(oc + 1) * 128],
                                         localT[:, ic, j * 512:(j + 1) * 512],
                                         start=False, stop=False)
                    nc.tensor.matmul(ps[:], ident_r[:],
                                     xqT[:, oc, 4 + j * 512: 4 + j * 512 + 512],
                                     start=False, stop=True)
                    nc.scalar.activation(hT[:, oc, j * 512:(j + 1) * 512], ps[:],
                                         AF.Identity, bias=dbias[:, oc:oc + 1])
                ssum = psL.tile([1, 512], f32, tag="lns")
                ssq = psL.tile([1, 512], f32, tag="lnq")
                for oc in range(NCH):
                    nc.tensor.matmul(ssum[:], ones_col[:], hT[:, oc, j * 512:(j + 1) * 512],
                                     start=(oc == 0), stop=(oc == NCH - 1))
                for oc in range(NCH):
                    hsq = pcw.tile([128, 512], f32r, tag="hsq")
                    nc.scalar.activation(hsq[:], hT[:, oc, j * 512:(j + 1) * 512], AF.Square)
                    nc.tensor.matmul(ssq[:], ones_col[:], hsq[:],
                                     start=(oc == 0), stop=(oc == NCH - 1))
                mu = pcw.tile([1, 512], f32, tag="mu")
                nc.vector.tensor_scalar(mu[:], ssum[:], 1.0 / D, None, op0=MUL)
                var = pcw.tile([1, 512], f32, tag="var")
                nc.vector.tensor_scalar(var[:], ssq[:], 1.0 / D, None, op0=MUL)
                msq = pcw.tile([1, 512], f32, tag="msq")
                nc.vector.tensor_mul(msq[:], mu[:], mu[:])
                nc.vector.tensor_sub(var[:], var[:], msq[:])
                sd = pcw.tile([1, 512], f32, tag="sd")
                nc.scalar.activation(sd[:], var[:], AF.Sqrt, bias=eps_t[:])
                rstd = pcw.tile([1, 512], f32, tag="rstd")
                nc.vector.reciprocal(rstd[:], sd[:])
                ar = pcw.tile([1, 512], f32r, tag="ar")
                nc.vector.tensor_copy(ar[:], rstd[:])
                br = pcw.tile([1, 512], f32, tag="br")
                nc.vector.tensor_mul(br[:], mu[:], rstd[:])
                brr = pcw.tile([1, 512], f32r, tag="brr")
                nc.vector.tensor_scalar(brr[:], br[:], -1.0, None, op0=MUL)
                for oc in range(NCH):
                    abc = psL.tile([128, 512], f32, tag="abc")
                    nc.tensor.matmul(abc[:], grow[:, oc * 128:(oc + 1) * 128], ar[:],
                                     start=True, stop=True)
                    bbc = psL.tile([128, 512], f32, tag="bbc")
                    nc.tensor.matmul(bbc[:], grow[:, oc * 128:(oc + 1) * 128], brr[:],
                                     start=True, stop=False)
                    nc.tensor.matmul(bbc[:], brow[:, oc * 128:(oc + 1) * 128], ones_row[:],
                                     start=False, stop=True)
                    t1 = pcw.tile([128, 512], f32, tag="t1")
                    nc.vector.tensor_mul(t1[:], hT[:, oc, j * 512:(j + 1) * 512], abc[:])
                    nc.vector.tensor_add(outT[:, oc, j * 512:(j + 1) * 512], t1[:], bbc[:])
            nc.sync.dma_start(out_v[:, :, :], outT[:])

    nc.compile()
    return nc


class _Runner:
    """Persistent-jit SPMD executor (replicates bass2jax.run_bass_via_pjrt)."""

    def __init__(self, nc, n_cores=8):
        import jax
        from jax.sharding import Mesh, PartitionSpec
        from jax.experimental.shard_map import shard_map
        from concourse import mybir
        from concourse.bass2jax import (_bass_exec_p, install_neuronx_cc_hook,
                                        partition_id_tensor)
        install_neuronx_cc_hook()
        self.jax = jax
        self.n_cores = n_cores
        partition_name = nc.partition_id_tensor.name if nc.partition_id_tensor else None
        in_names, out_names, out_avals, zero_outs = [], [], [], []
        for alloc in nc.m.functions[0].allocations:
            if not isinstance(alloc, mybir.MemoryLocationSet):
                continue
            name = alloc.memorylocations[0].name
            if alloc.kind == "ExternalInput":
                if name != partition_name:
                    in_names.append(name)
            elif alloc.kind == "ExternalOutput":
                out_names.append(name)
                shape = tuple(alloc.tensor_shape)
                dtype = mybir.dt.np(alloc.dtype)
                out_avals.append(jax.core.ShapedArray(shape, dtype))
                zero_outs.append(np.zeros(shape, dtype))
        self.in_names, self.out_names = in_names, out_names
        self.out_avals, self.zero_outs = out_avals, zero_outs
        all_in_names = list(in_names) + list(out_names)
        if partition_name is not None:
            all_in_names.append(partition_name)

        def _body(*args):
            operands = list(args)
            if partition_name is not None:
                operands.append(partition_id_tensor())
            outs = _bass_exec_p.bind(
                *operands, out_avals=tuple(out_avals), in_names=tuple(all_in_names),
                out_names=tuple(out_names), lowering_input_output_aliases=(),
                sim_require_finite=True, sim_require_nnan=True, nc=nc)
            return tuple(outs)

        devices = jax.devices()[:n_cores]
        mesh = Mesh(np.asarray(devices), ("core",))
        n_tot = len(in_names) + len(out_names)
        self.fn = jax.jit(
            shard_map(_body, mesh=mesh, in_specs=(PartitionSpec("core"),) * n_tot,
                      out_specs=(PartitionSpec("core"),) * len(out_names),
                      check_rep=False),
            keep_unused=True)

    def put_inputs(self, in_maps):
        per_core = [[np.asarray(m[name]) for name in self.in_names] for m in in_maps]
        concat_in = [np.concatenate([per_core[c][i] for c in range(self.n_cores)], axis=0)
                     for i in range(len(self.in_names))]
        concat_zeros = [np.zeros((self.n_cores * z.shape[0], *z.shape[1:]), z.dtype)
                        for z in self.zero_outs]
        return [self.jax.device_put(a) for a in concat_in + concat_zeros]

    def run(self, args):
        outs = self.fn(*args)
        self.jax.block_until_ready(outs)
        return outs

    def results(self, outs):
        return [
            {name: np.asarray(outs[i]).reshape(self.n_cores, *self.out_avals[i].shape)[c]
             for i, name in enumerate(self.out_names)}
            for c in range(self.n_cores)
        ]


def _get_runner():
    if "runner" not in _CACHE:
        nc = _build()
        _CACHE["runner"] = _Runner(nc, 8)
    return _CACHE["runner"]


def _make_constants():
    import ml_dtypes
    ek = np.zeros((KER, C3, HK, 128), np.float32)
    for k in range(KER):
        for c in range(C3):
            for hh in range(2):
                h = 2 * c + hh
                ek[k, c, h * KER + k, hh * 64:(hh + 1) * 64] = 1.0
    ssum = np.zeros((HK, H), np.float32)
    sbc = np.zeros((H, HK), np.float32)
    for h in range(H):
        ssum[h * KER:(h + 1) * KER, h] = 1.0
        sbc[h, h * KER:(h + 1) * KER] = 1.0
    bf = ml_dtypes.bfloat16
    return (ek.astype(bf), ssum.astype(bf), sbc.astype(bf),
            np.eye(128, dtype=np.float32))


def kernel(**inputs):
    import ml_dtypes
    x = np.asarray(inputs["x"], np.float32)
    mask = np.asarray(inputs["mask"], np.int32)
    ek, ssum, sbc, ident = _make_constants()
    common = {nm: np.asarray(inputs[nm], np.float32) for nm in
              ["wq_w", "wk_w", "wv_w", "cc_pw", "co_w", "dense_w", "cc_dw",
               "wq_b", "wk_b", "wv_b", "cc_b", "co_b", "ck_b", "dense_b",
               "gamma", "beta"]}
    common["ck_w"] = np.asarray(inputs["ck_w"], np.float32).astype(ml_dtypes.bfloat16)
    common.update({"ek": ek, "ssum": ssum, "sbc": sbc, "ident": ident,
                   "identr": ident})

    in_maps = []
    for core in range(8):
        b, half = core // 2, core % 2
        q0 = half * LH
        xq = np.zeros((XQR, D), np.float32)
        lo, hi = q0 - 4, q0 - 4 + XQR
        slo, shi = max(lo, 0), min(hi, L)
        xq[slo - lo: shi - lo] = x[b, slo:shi]
        m = dict(common)
        m["xf"] = np.ascontiguousarray(x[b])
        m["xq"] = xq
        m["mask"] = np.ascontiguousarray(mask[b])
        in_maps.append(m)

    r = _get_runner()
    args = r.put_inputs(in_maps)
    outs = r.run(args)
    res = r.results(outs)

    out = np.empty((B, L, D), np.float32)
    a1 = np.empty((B, H, L, L), np.float32)
    for core in range(8):
        b, half = core // 2, core % 2
        q0 = half * LH
        out[b, q0:q0 + LH, :] = res[core]["outt"].T
        a1[b, :, q0:q0 + LH, :] = res[core]["a1t"].transpose(0, 2, 1)
    return out, a1
